# revision 25
# baseline (speedup 1.0000x reference)
"""Trainium2 Bass kernel for an attentive LSTM cell.

Data-parallel across 8 NeuronCores: batch (64) sharded 8 rows/core, weights
replicated.  Per core:

  - annotations are cast fp32->bf16 inside the DMA (SWDGE) as two half-row
    transfers per batch row, kept resident in SBUF, and prefetched two rows
    ahead so the PE never waits on HBM (an idle PE also re-throttles the
    HAM clock gate, doubling matmul cost — prefetch depth matters twice).
  - ann^T is built with regular bf16 matmuls against an identity moving
    operand (not transpose-mode, which the HAM activity monitor ignores),
    staged through PSUM and cast bf16->fp8e4 on DVE.
  - uh^T = ku^T @ ann^T runs as fp8 DoubleRow matmuls (ku pre-scaled by 64
    into fp8's normal range; the tanh activation un-scales via its input
    scale), halving PE streaming vs bf16.
  - tanh batches per 128-unit chunk over a 1024-wide half row (one
    per-partition bias column per chunk); output is fp8, so
    et = v . tanh(...) also runs as DoubleRow matmuls.
  - per-row softmax uses exp with an in-instruction denominator
    accumulator; context = sum_q w_col[q] . ann[q] over the resident row,
    normalized once at the end.
  - et/exp/w-cols/context of row b-1 are issued between the two half-row
    blocks of row b (one-row software pipeline), so the PE never stalls on
    the scalar engine.
  - LSTM tail weights (12 MB) load as fp32 on the HWDGE queue (parallel to
    the SWDGE queue carrying annotations), are converted to bf16 on DVE in
    chunks interleaved into the batch loop, and the tail runs batched bf16
    matmuls over the core's 8 rows.
"""

import os
import sys

for _p in ("/opt/trn_rl_repo", "/root/.axon_site/_ro/trn_rl_repo"):
    if os.path.isdir(_p) and _p not in sys.path:
        sys.path.insert(0, _p)

import numpy as np

import concourse.bass as bass
import concourse.mybir as mybir
import concourse.tile as tile
from concourse import bacc
from concourse.bass_utils import run_bass_kernel_spmd
from concourse.masks import make_identity

AF = mybir.ActivationFunctionType
DR = mybir.MatmulPerfMode.DoubleRow
F32 = mybir.dt.float32
F32R = mybir.dt.float32r
BF16 = mybir.dt.bfloat16
FP8 = mybir.dt.float8e4

ANN_FP8 = False        # fp8 resident annotations + DoubleRow context
                       # (fp32->fp8 cast-DMA measured slower on HW than bf16)
KU_SCALE = 64.0        # ku pre-scale before fp8 cast (values ~N(0, 0.02))
V_SCALE = 64.0         # kv pre-scale before fp8 cast

N_CORES = 8
B, T, A, U, D = 64, 2048, 512, 512, 512
BS = B // N_CORES  # batch rows per core
TT = 512           # t macro-tile
NT = T // TT       # macro tiles per batch row
NS = TT // 128     # 128-row subtiles per macro tile
J = A // 128       # contraction chunks (annotation dim)
M = U // 128       # unit chunks
TS = T // 128      # 128-row subtiles per full batch row
HB = 1024          # tanh half-row width


def build_bass(stage="full", repeat=1):
    nc = bacc.Bacc(trn_type="TRN2", debug=False)

    ann_d = nc.dram_tensor("ann", [BS, T, A], F32, kind="ExternalInput").ap()
    inp_d = nc.dram_tensor("inputs", [BS, D], F32, kind="ExternalInput").ap()
    h_d = nc.dram_tensor("h", [BS, U], F32, kind="ExternalInput").ap()
    c_d = nc.dram_tensor("c", [BS, U], F32, kind="ExternalInput").ap()
    W_d = nc.dram_tensor("kernel", [D + A, 4 * U], F32, kind="ExternalInput").ap()
    R_d = nc.dram_tensor("rkernel", [U, 4 * U], F32, kind="ExternalInput").ap()
    bias_d = nc.dram_tensor("bias", [1, 6 * U], F32, kind="ExternalInput").ap()
    ku_d = nc.dram_tensor("ku", [A, U], F32, kind="ExternalInput").ap()
    kw_d = nc.dram_tensor("kw", [U, U], F32, kind="ExternalInput").ap()
    kv_d = nc.dram_tensor("kv", [1, U], F32, kind="ExternalInput").ap()
    out_d = nc.dram_tensor("out", [BS, U], F32, kind="ExternalOutput").ap()

    with tile.TileContext(nc) as tc:
        if repeat > 1:
            with tc.For_i(0, repeat, 1):
                _body(nc, tc, ann_d, inp_d, h_d, c_d, W_d, R_d, bias_d, ku_d,
                      kw_d, kv_d, out_d)
        else:
            _body(nc, tc, ann_d, inp_d, h_d, c_d, W_d, R_d, bias_d, ku_d,
                  kw_d, kv_d, out_d)
    nc.compile()
    return nc


def _body(nc, tc, ann_d, inp_d, h_d, c_d, W_d, R_d, bias_d, ku_d, kw_d, kv_d,
          out_d):
    ANT = FP8 if ANN_FP8 else BF16   # resident annotation dtype
    HQ = TS // 2  # row-half in 128-subtiles
    with (
        tc.tile_pool(name="const", bufs=1) as cpool,
        tc.tile_pool(name="wts", bufs=1) as wpool,
        tc.tile_pool(name="annres", bufs=3) as annpool,
        tc.tile_pool(name="annT", bufs=2) as annTpool,
        tc.tile_pool(name="tanh", bufs=2) as tanhpool,
        tc.tile_pool(name="wstage", bufs=2) as wstpool,
        tc.tile_pool(name="small_sb", bufs=2) as smallsb,
    ):
        def issue_ann_dma(b):
            halves = []
            for h in range(2):
                ah = annpool.tile([128, HQ, A], ANT, tag=f"annres{h}")
                nc.gpsimd.dma_start(
                    out=ah,
                    in_=ann_d[b, T // 2 * h:T // 2 * (h + 1), :]
                    .rearrange("(q p) a -> p q a", p=128))
                halves.append(ah)
            return halves

        # identity first: make_identity runs on the gpsimd/Pool queue, and it
        # must not sit behind the annotation DMAs issued onto the same queue
        # (every PE transpose depends on it).
        ident = cpool.tile([128, 128], F32)
        make_identity(nc, ident)
        ident_t = cpool.tile([128, 128], ANT)
        nc.vector.tensor_copy(ident_t, ident)

        # annotation DMAs for the first two rows start before everything else
        ann_q = [issue_ann_dma(0), issue_ann_dma(1)]
        ones11 = cpool.tile([1, 1], F32)
        nc.vector.memset(ones11, 1.0)
        ones11_t = cpool.tile([1, 1], ANT)
        nc.vector.tensor_copy(ones11_t, ones11)
        ones1b_ld = cpool.tile([1, BS], F32)
        nc.vector.memset(ones1b_ld, 1.0)
        ones1b = cpool.tile([1, BS], BF16)
        nc.vector.tensor_copy(ones1b, ones1b_ld)
        half_col = cpool.tile([BS, 1], F32)
        nc.vector.memset(half_col, 0.5)

        # --- replicated weights (fp32 loaders in a transient pool) ---
        ldpool_cm = tc.tile_pool(name="ld", bufs=1)
        ldpool = ldpool_cm.__enter__()
        ku_ld = ldpool.tile([128, J, U], F32)  # ku[a, u] -> [p, j, u], a=128j+p
        nc.sync.dma_start(out=ku_ld, in_=ku_d.rearrange("(j p) u -> p j u", p=128))
        ku_sc = ldpool.tile([128, J, U], F32, name="ku_sc")
        nc.vector.tensor_scalar_mul(ku_sc, ku_ld, KU_SCALE)
        ku_sb = wpool.tile([128, J, U], FP8)
        nc.vector.tensor_copy(ku_sb, ku_sc)
        kw_ld = ldpool.tile([128, J, U], F32)
        nc.sync.dma_start(out=kw_ld, in_=kw_d.rearrange("(j p) u -> p j u", p=128))
        kw_sb = wpool.tile([128, J, U], BF16)
        nc.vector.tensor_copy(kw_sb, kw_ld)
        v_ld = cpool.tile([128, M], F32)       # v[u] -> [p, m], u=128m+p
        nc.sync.dma_start(out=v_ld, in_=kv_d.rearrange("o (m p) -> p (o m)", p=128))
        v_sc = cpool.tile([128, M], F32)
        nc.vector.tensor_scalar_mul(v_sc, v_ld, V_SCALE)
        v_pad = cpool.tile([128, M, 16], FP8)  # fp8 v, 16B-padded k-tile step
        nc.vector.memset(v_pad, 0.0)
        nc.vector.tensor_copy(v_pad[:, :, 0], v_sc)
        biasu_col = cpool.tile([128, M], F32)  # bias[4U:5U] as a column
        nc.sync.dma_start(
            out=biasu_col,
            in_=bias_d[:, 4 * U:5 * U].rearrange("o (m p) -> p (o m)", p=128))
        biasz_ld = cpool.tile([1, 4 * U], F32)
        nc.sync.dma_start(out=biasz_ld, in_=bias_d[:, 0:4 * U])
        biasz_row = cpool.tile([1, 4 * U], BF16)
        nc.vector.tensor_copy(biasz_row, biasz_ld)

        # --- per-core state rows ---
        h_nat = cpool.tile([BS, U], F32)
        nc.sync.dma_start(out=h_nat, in_=h_d)
        in_nat = cpool.tile([BS, D], F32)
        nc.sync.dma_start(out=in_nat, in_=inp_d)
        c_nat = cpool.tile([BS, U], F32)
        nc.sync.dma_start(out=c_nat, in_=c_d)

        hT = wpool.tile([128, M, BS], BF16)      # h^T, contraction layout
        xT = wpool.tile([128, 2 * J, BS], BF16)  # [inputs; context]^T
        bias_att = wpool.tile([128, M, BS], F32)  # Wx^T + bias_u per batch row

        with tc.tile_pool(name="ps_setup", bufs=2, space="PSUM") as pps:
            for j in range(M):
                pt = pps.tile([128, BS], F32)
                nc.tensor.transpose(pt, h_nat[:, 128 * j:128 * (j + 1)],
                                    ident[0:BS, 0:BS])
                nc.vector.tensor_copy(hT[:, j, :], pt)
            for j in range(J):
                pt = pps.tile([128, BS], F32)
                nc.tensor.transpose(pt, in_nat[:, 128 * j:128 * (j + 1)],
                                    ident[0:BS, 0:BS])
                nc.vector.tensor_copy(xT[:, j, :], pt)
            for m in range(M):
                pwx = pps.tile([128, BS], F32)
                for j in range(M):
                    nc.tensor.matmul(pwx,
                                     lhsT=kw_sb[:, j, 128 * m:128 * (m + 1)],
                                     rhs=hT[:, j, :],
                                     start=(j == 0), stop=(j == M - 1))
                nc.scalar.activation(bias_att[:, m, :], pwx, AF.Identity,
                                     bias=biasu_col[:, m:m + 1])
        ldpool_cm.__exit__(None, None, None)

        # LSTM tail weights: fp32 loads on the HWDGE queue (parallel to the
        # Pool/SWDGE queue carrying annotations), converted to bf16 on DVE.
        Wt = wpool.tile([128, 2 * J, 4 * U], BF16)
        Rt = wpool.tile([128, M, 4 * U], BF16)
        w_chunks = []
        for n in range(4):
            for kh in range(2):
                w_chunks.append((Wt[:, 4 * kh:4 * (kh + 1), U * n:U * (n + 1)],
                                 W_d[512 * kh:512 * (kh + 1),
                                     U * n:U * (n + 1)].rearrange(
                                     "(k p) n -> p k n", p=128)))
            w_chunks.append((Rt[:, :, U * n:U * (n + 1)],
                             R_d[:, U * n:U * (n + 1)].rearrange(
                                 "(k p) n -> p k n", p=128)))

        def issue_w_chunk(c):
            dst, src = w_chunks[c]
            st = wstpool.tile([128, M, U], F32, tag="wst")
            nc.sync.dma_start(out=st, in_=src)
            nc.vector.tensor_copy(dst, st)

        # ------------- attention -------------
        with (
            tc.tile_pool(name="stg_ps", bufs=2, space="PSUM") as stgps,
            tc.tile_pool(name="uh_ps", bufs=2, space="PSUM") as uhps,
            tc.tile_pool(name="small_ps", bufs=2, space="PSUM") as smallps,
        ):
            pend = None

            def late_stage(p):
                # et / exp / w-cols / context for batch row b (one row late)
                b, tanhG, ann_halves = p
                denb = smallsb.tile([1, NT], F32, tag="den")
                w_cols = smallsb.tile([128, TS, 16], ANT, tag="wcols")
                for i in range(NT):
                    et_ps = smallps.tile([1, TT], F32, tag="sm")
                    for g in range(M // 2):
                        nc.tensor.matmul(
                            et_ps, lhsT=v_pad[:, 2 * g:2 * g + 2, 0:1],
                            rhs=tanhG[:, 2 * g:2 * g + 2,
                                      TT * i:TT * (i + 1)],
                            start=(g == 0), stop=(g == M // 2 - 1),
                            perf_mode=DR)
                    w_row = smallsb.tile([1, TT], ANT, tag="wrow")
                    nc.scalar.activation(w_row, et_ps, AF.Exp,
                                         scale=1.0 / V_SCALE,
                                         accum_out=denb[:, i:i + 1])
                    wc_ps = smallps.tile([128, NS * 4], ANT, tag="sm")
                    wcw = 4 if ANN_FP8 else 2  # pad cols to 4 bytes
                    for s in range(NS):
                        nc.tensor.transpose(wc_ps[:, wcw * s:wcw * s + 1],
                                            w_row[:, 128 * s:128 * (s + 1)],
                                            ones11_t)
                    nc.vector.tensor_copy(
                        w_cols[:, NS * i:NS * (i + 1), 0],
                        wc_ps.rearrange("p (s w) -> p s w", w=wcw)[:, 0:NS, 0])
                dsum = smallsb.tile([1, 1], F32, tag="dsum")
                nc.vector.reduce_sum(dsum, denb, axis=mybir.AxisListType.X)
                drec = smallsb.tile([1, 1], F32, tag="drec")
                nc.vector.reciprocal(drec, dsum)
                ctx_ps = smallps.tile([1, A], F32, tag="sm")
                if ANN_FP8:
                    for h in range(2):
                        for g in range(HQ // 2):
                            nc.tensor.matmul(
                                ctx_ps,
                                lhsT=w_cols[:, HQ * h + 2 * g:
                                            HQ * h + 2 * g + 2, 0:1],
                                rhs=ann_halves[h][:, 2 * g:2 * g + 2, :],
                                start=(h == 0 and g == 0),
                                stop=(h == 1 and g == HQ // 2 - 1),
                                perf_mode=DR)
                else:
                    for q in range(TS):
                        nc.tensor.matmul(
                            ctx_ps, lhsT=w_cols[:, q, 0:1],
                            rhs=ann_halves[q // HQ][:, q % HQ, :],
                            start=(q == 0), stop=(q == TS - 1))
                ctx_row = smallsb.tile([1, A], F32, tag="ctxrow")
                nc.vector.tensor_scalar_mul(ctx_row, ctx_ps, drec)
                cT_ps = smallps.tile([128, J], F32, tag="sm")
                for j in range(J):
                    nc.tensor.transpose(cT_ps[:, j:j + 1],
                                        ctx_row[:, 128 * j:128 * (j + 1)],
                                        ones11)
                nc.vector.tensor_copy(xT[:, J:2 * J, b], cT_ps)

            for b in range(BS):
                ann_halves = ann_q.pop(0)
                if b + 2 < BS:
                    ann_q.append(issue_ann_dma(b + 2))
                for c in (2 * b, 2 * b + 1):
                    if c < len(w_chunks):
                        issue_w_chunk(c)

                # Per half-row: transpose burst then uh/tanh burst, so plain
                # matmuls pulse on PE at a sub-3.4us cadence (HAM warmth).
                # Transposes are REGULAR bf16 matmuls (out = ann_chunk.T @ I)
                # rather than transpose-mode, which the HAM activity monitor
                # does not count as PE-busy.
                annT = annTpool.tile([128, J, T], FP8)
                tanhG = tanhpool.tile([128, M, T], FP8)
                for h in range(2):
                    for i in range(HQ // NS):
                        for j in range(J):
                            stg = stgps.tile([128, TT], F32, tag="stg")
                            for s in range(NS):
                                nc.tensor.matmul(
                                    stg[:, 128 * s:128 * (s + 1)],
                                    lhsT=ann_halves[h][:, NS * i + s,
                                                       128 * j:128 * (j + 1)],
                                    rhs=ident_t,
                                    start=True, stop=True)
                            nc.vector.tensor_copy(
                                annT[:, j, T // 2 * h + TT * i:
                                     T // 2 * h + TT * (i + 1)], stg)
                    for m in range(M):
                        uh = uhps.tile([128, HB], F32, tag="uh")
                        for c in range(2):
                            for g in range(J // 2):
                                nc.tensor.matmul(
                                    uh[:, TT * c:TT * (c + 1)],
                                    lhsT=ku_sb[:, 2 * g:2 * g + 2,
                                               128 * m:128 * (m + 1)],
                                    rhs=annT[:, 2 * g:2 * g + 2,
                                             HB * h + TT * c:
                                             HB * h + TT * (c + 1)],
                                    start=(g == 0), stop=(g == J // 2 - 1),
                                    perf_mode=DR)
                        nc.scalar.activation(
                            tanhG[:, m, HB * h:HB * (h + 1)], uh, AF.Tanh,
                            bias=bias_att[:, m, b:b + 1],
                            scale=1.0 / KU_SCALE)
                    if h == 0 and pend is not None:
                        late_stage(pend)
                        pend = None

                pend = (b, tanhG, ann_halves)

            late_stage(pend)

        # ------------- LSTM tail -------------
        with (
            tc.tile_pool(name="z_ps", bufs=2, space="PSUM") as zpool,
            tc.tile_pool(name="gates", bufs=1) as gpool,
        ):
            gates = []
            for n in range(4):
                zps = zpool.tile([BS, U], F32)
                for k in range(2 * J):
                    nc.tensor.matmul(zps, lhsT=xT[:, k, :],
                                     rhs=Wt[:, k, U * n:U * (n + 1)],
                                     start=(k == 0), stop=False)
                for k in range(M):
                    nc.tensor.matmul(zps, lhsT=hT[:, k, :],
                                     rhs=Rt[:, k, U * n:U * (n + 1)],
                                     start=False, stop=False)
                nc.tensor.matmul(zps, lhsT=ones1b,
                                 rhs=biasz_row[:, U * n:U * (n + 1)],
                                 start=False, stop=True)
                g = gpool.tile([BS, U], F32, tag=f"gate{n}")
                if n == 2:
                    nc.scalar.activation(g, zps, AF.Tanh)
                else:
                    nc.scalar.activation(g, zps, AF.Relu, bias=half_col,
                                         scale=0.2)
                    nc.vector.tensor_scalar_min(g, g, 1.0)
                gates.append(g)

            gi, gf, gg, go = gates
            c_new = gpool.tile([BS, U], F32, tag="cnew")
            nc.vector.tensor_mul(c_new, gf, c_nat)
            ig = gpool.tile([BS, U], F32, tag="ig")
            nc.vector.tensor_mul(ig, gi, gg)
            nc.vector.tensor_add(c_new, c_new, ig)
            tc_t = gpool.tile([BS, U], F32, tag="tanhc")
            nc.scalar.activation(tc_t, c_new, AF.Tanh)
            h_new = gpool.tile([BS, U], F32, tag="hnew")
            nc.vector.tensor_mul(h_new, go, tc_t)
            nc.sync.dma_start(out=out_d, in_=h_new)


_NC_CACHE = None


def _get_nc():
    global _NC_CACHE
    if _NC_CACHE is None:
        _NC_CACHE = build_bass()
    return _NC_CACHE


def make_in_maps(inputs, h, c, annotations, kernel, recurrent_kernel, bias,
                 kernel_u, kernel_w, kernel_v):
    asc = np.ascontiguousarray
    maps = []
    for core in range(N_CORES):
        sl = slice(core * BS, (core + 1) * BS)
        maps.append({
            "ann": asc(annotations[sl]).astype(np.float32),
            "inputs": asc(inputs[sl]).astype(np.float32),
            "h": asc(h[sl]).astype(np.float32),
            "c": asc(c[sl]).astype(np.float32),
            "kernel": asc(kernel).astype(np.float32),
            "rkernel": asc(recurrent_kernel).astype(np.float32),
            "bias": asc(bias).reshape(1, 6 * U).astype(np.float32),
            "ku": asc(kernel_u).astype(np.float32),
            "kw": asc(kernel_w).astype(np.float32),
            "kv": asc(kernel_v).reshape(1, U).astype(np.float32),
        })
    return maps


def kernel(inputs, h, c, annotations, kernel, recurrent_kernel, bias,
           kernel_u, kernel_w, kernel_v, _trace=False):
    nc = _get_nc()
    in_maps = make_in_maps(inputs, h, c, annotations, kernel,
                           recurrent_kernel, bias, kernel_u, kernel_w,
                           kernel_v)
    res = run_bass_kernel_spmd(nc, in_maps, list(range(N_CORES)),
                               trace=_trace)
    out = np.concatenate([res.results[i]["out"] for i in range(N_CORES)],
                         axis=0)
    if _trace:
        globals()["last_exec_time_ns"] = res.exec_time_ns
        globals()["last_results"] = res
    return out


# revision 26
# speedup vs baseline: 1.0730x; 1.0730x over previous
"""Trainium2 Bass kernel for an attentive LSTM cell.

Data-parallel across 8 NeuronCores: batch (64) sharded 8 rows/core, weights
replicated.  Per core:

  - annotations are cast fp32->bf16 inside the DMA (SWDGE) as two half-row
    transfers per batch row, kept resident in SBUF, and prefetched two rows
    ahead so the PE never waits on HBM (an idle PE also re-throttles the
    HAM clock gate, doubling matmul cost — prefetch depth matters twice).
  - ann^T is built with regular bf16 matmuls against an identity moving
    operand (not transpose-mode, which the HAM activity monitor ignores),
    staged through PSUM and cast bf16->fp8e4 on DVE.
  - uh^T = ku^T @ ann^T runs as fp8 DoubleRow matmuls (ku pre-scaled by 64
    into fp8's normal range; the tanh activation un-scales via its input
    scale), halving PE streaming vs bf16.
  - tanh batches per 128-unit chunk over a 1024-wide half row (one
    per-partition bias column per chunk); output is fp8, so
    et = v . tanh(...) also runs as DoubleRow matmuls.
  - per-row softmax uses exp with an in-instruction denominator
    accumulator; context = sum_q w_col[q] . ann[q] over the resident row,
    normalized once at the end.
  - et/exp/w-cols/context of row b-1 are issued between the two half-row
    blocks of row b (one-row software pipeline), so the PE never stalls on
    the scalar engine.
  - LSTM tail weights (12 MB) load as fp32 on the HWDGE queue (parallel to
    the SWDGE queue carrying annotations), are converted to bf16 on DVE in
    chunks interleaved into the batch loop, and the tail runs batched bf16
    matmuls over the core's 8 rows.
"""

import os
import sys

for _p in ("/opt/trn_rl_repo", "/root/.axon_site/_ro/trn_rl_repo"):
    if os.path.isdir(_p) and _p not in sys.path:
        sys.path.insert(0, _p)

import numpy as np

import concourse.bass as bass
import concourse.mybir as mybir
import concourse.tile as tile
from concourse import bacc
from concourse.bass_utils import run_bass_kernel_spmd
from concourse.masks import make_identity

AF = mybir.ActivationFunctionType
DR = mybir.MatmulPerfMode.DoubleRow
F32 = mybir.dt.float32
F32R = mybir.dt.float32r
BF16 = mybir.dt.bfloat16
FP8 = mybir.dt.float8e4

ANN_FP8 = False        # fp8 resident annotations + DoubleRow context
                       # (fp32->fp8 cast-DMA measured slower on HW than bf16)
KU_SCALE = 64.0        # ku pre-scale before fp8 cast (values ~N(0, 0.02))
V_SCALE = 64.0         # kv pre-scale before fp8 cast

N_CORES = 8
B, T, A, U, D = 64, 2048, 512, 512, 512
BS = B // N_CORES  # batch rows per core
TT = 512           # t macro-tile
NT = T // TT       # macro tiles per batch row
NS = TT // 128     # 128-row subtiles per macro tile
J = A // 128       # contraction chunks (annotation dim)
M = U // 128       # unit chunks
TS = T // 128      # 128-row subtiles per full batch row
HB = 1024          # tanh half-row width


def build_bass(stage="full", repeat=1):
    nc = bacc.Bacc(trn_type="TRN2", debug=False)

    ann_d = nc.dram_tensor("ann", [BS, T, A], F32, kind="ExternalInput").ap()
    inp_d = nc.dram_tensor("inputs", [BS, D], F32, kind="ExternalInput").ap()
    h_d = nc.dram_tensor("h", [BS, U], F32, kind="ExternalInput").ap()
    c_d = nc.dram_tensor("c", [BS, U], F32, kind="ExternalInput").ap()
    W_d = nc.dram_tensor("kernel", [D + A, 4 * U], F32, kind="ExternalInput").ap()
    R_d = nc.dram_tensor("rkernel", [U, 4 * U], F32, kind="ExternalInput").ap()
    bias_d = nc.dram_tensor("bias", [1, 6 * U], F32, kind="ExternalInput").ap()
    ku_d = nc.dram_tensor("ku", [A, U], F32, kind="ExternalInput").ap()
    kw_d = nc.dram_tensor("kw", [U, U], F32, kind="ExternalInput").ap()
    kv_d = nc.dram_tensor("kv", [1, U], F32, kind="ExternalInput").ap()
    out_d = nc.dram_tensor("out", [BS, U], F32, kind="ExternalOutput").ap()

    with tile.TileContext(nc) as tc:
        if repeat > 1:
            with tc.For_i(0, repeat, 1):
                _body(nc, tc, ann_d, inp_d, h_d, c_d, W_d, R_d, bias_d, ku_d,
                      kw_d, kv_d, out_d)
        else:
            _body(nc, tc, ann_d, inp_d, h_d, c_d, W_d, R_d, bias_d, ku_d,
                  kw_d, kv_d, out_d)
    nc.compile()
    return nc


def _body(nc, tc, ann_d, inp_d, h_d, c_d, W_d, R_d, bias_d, ku_d, kw_d, kv_d,
          out_d):
    ANT = FP8 if ANN_FP8 else BF16   # resident annotation dtype
    HQ = TS // 2  # row-half in 128-subtiles
    with (
        tc.tile_pool(name="const", bufs=1) as cpool,
        tc.tile_pool(name="wts", bufs=1) as wpool,
        tc.tile_pool(name="annres", bufs=3) as annpool,
        tc.tile_pool(name="annT", bufs=2) as annTpool,
        tc.tile_pool(name="tanh", bufs=2) as tanhpool,
        tc.tile_pool(name="wstage", bufs=2) as wstpool,
        tc.tile_pool(name="small_sb", bufs=2) as smallsb,
    ):
        def issue_ann_dma(b):
            halves = []
            for h in range(2):
                ah = annpool.tile([128, HQ, A], ANT, tag=f"annres{h}")
                nc.gpsimd.dma_start(
                    out=ah,
                    in_=ann_d[b, T // 2 * h:T // 2 * (h + 1), :]
                    .rearrange("(q p) a -> p q a", p=128))
                halves.append(ah)
            return halves

        # annotation DMAs for the first two rows start before anything else
        ann_q = [issue_ann_dma(0), issue_ann_dma(1)]

        ident = cpool.tile([128, 128], F32)
        make_identity(nc, ident)
        ident_t = cpool.tile([128, 128], ANT)
        nc.vector.tensor_copy(ident_t, ident)
        ones11 = cpool.tile([1, 1], F32)
        nc.vector.memset(ones11, 1.0)
        ones11_t = cpool.tile([1, 1], ANT)
        nc.vector.tensor_copy(ones11_t, ones11)
        ones1b_ld = cpool.tile([1, BS], F32)
        nc.vector.memset(ones1b_ld, 1.0)
        ones1b = cpool.tile([1, BS], BF16)
        nc.vector.tensor_copy(ones1b, ones1b_ld)
        half_col = cpool.tile([BS, 1], F32)
        nc.vector.memset(half_col, 0.5)

        # --- replicated weights (fp32 loaders in a transient pool) ---
        ldpool_cm = tc.tile_pool(name="ld", bufs=1)
        ldpool = ldpool_cm.__enter__()
        ku_ld = ldpool.tile([128, J, U], F32)  # ku[a, u] -> [p, j, u], a=128j+p
        nc.sync.dma_start(out=ku_ld, in_=ku_d.rearrange("(j p) u -> p j u", p=128))
        ku_sc = ldpool.tile([128, J, U], F32, name="ku_sc")
        nc.vector.tensor_scalar_mul(ku_sc, ku_ld, KU_SCALE)
        ku_sb = wpool.tile([128, J, U], FP8)
        nc.vector.tensor_copy(ku_sb, ku_sc)
        kw_ld = ldpool.tile([128, J, U], F32)
        nc.sync.dma_start(out=kw_ld, in_=kw_d.rearrange("(j p) u -> p j u", p=128))
        kw_sb = wpool.tile([128, J, U], BF16)
        nc.vector.tensor_copy(kw_sb, kw_ld)
        v_ld = cpool.tile([128, M], F32)       # v[u] -> [p, m], u=128m+p
        nc.sync.dma_start(out=v_ld, in_=kv_d.rearrange("o (m p) -> p (o m)", p=128))
        v_sc = cpool.tile([128, M], F32)
        nc.vector.tensor_scalar_mul(v_sc, v_ld, V_SCALE)
        v_pad = cpool.tile([128, M, 16], FP8)  # fp8 v, 16B-padded k-tile step
        nc.vector.memset(v_pad, 0.0)
        nc.vector.tensor_copy(v_pad[:, :, 0], v_sc)
        biasu_col = cpool.tile([128, M], F32)  # bias[4U:5U] as a column
        nc.sync.dma_start(
            out=biasu_col,
            in_=bias_d[:, 4 * U:5 * U].rearrange("o (m p) -> p (o m)", p=128))
        biasz_ld = cpool.tile([1, 4 * U], F32)
        nc.sync.dma_start(out=biasz_ld, in_=bias_d[:, 0:4 * U])
        biasz_row = cpool.tile([1, 4 * U], BF16)
        nc.vector.tensor_copy(biasz_row, biasz_ld)

        # --- per-core state rows ---
        h_nat = cpool.tile([BS, U], F32)
        nc.sync.dma_start(out=h_nat, in_=h_d)
        in_nat = cpool.tile([BS, D], F32)
        nc.sync.dma_start(out=in_nat, in_=inp_d)
        c_nat = cpool.tile([BS, U], F32)
        nc.sync.dma_start(out=c_nat, in_=c_d)

        hT = wpool.tile([128, M, BS], BF16)      # h^T, contraction layout
        xT = wpool.tile([128, 2 * J, BS], BF16)  # [inputs; context]^T
        bias_att = wpool.tile([128, M, BS], F32)  # Wx^T + bias_u per batch row

        with tc.tile_pool(name="ps_setup", bufs=2, space="PSUM") as pps:
            for j in range(M):
                pt = pps.tile([128, BS], F32)
                nc.tensor.transpose(pt, h_nat[:, 128 * j:128 * (j + 1)],
                                    ident[0:BS, 0:BS])
                nc.vector.tensor_copy(hT[:, j, :], pt)
            for j in range(J):
                pt = pps.tile([128, BS], F32)
                nc.tensor.transpose(pt, in_nat[:, 128 * j:128 * (j + 1)],
                                    ident[0:BS, 0:BS])
                nc.vector.tensor_copy(xT[:, j, :], pt)
            for m in range(M):
                pwx = pps.tile([128, BS], F32)
                for j in range(M):
                    nc.tensor.matmul(pwx,
                                     lhsT=kw_sb[:, j, 128 * m:128 * (m + 1)],
                                     rhs=hT[:, j, :],
                                     start=(j == 0), stop=(j == M - 1))
                nc.scalar.activation(bias_att[:, m, :], pwx, AF.Identity,
                                     bias=biasu_col[:, m:m + 1])
        ldpool_cm.__exit__(None, None, None)

        # LSTM tail weights: fp32 loads on the HWDGE queue (parallel to the
        # Pool/SWDGE queue carrying annotations), converted to bf16 on DVE.
        Wt = wpool.tile([128, 2 * J, 4 * U], BF16)
        Rt = wpool.tile([128, M, 4 * U], BF16)
        w_chunks = []
        for n in range(4):
            for kh in range(2):
                w_chunks.append((Wt[:, 4 * kh:4 * (kh + 1), U * n:U * (n + 1)],
                                 W_d[512 * kh:512 * (kh + 1),
                                     U * n:U * (n + 1)].rearrange(
                                     "(k p) n -> p k n", p=128)))
            w_chunks.append((Rt[:, :, U * n:U * (n + 1)],
                             R_d[:, U * n:U * (n + 1)].rearrange(
                                 "(k p) n -> p k n", p=128)))

        def issue_w_chunk(c):
            dst, src = w_chunks[c]
            st = wstpool.tile([128, M, U], F32, tag="wst")
            nc.sync.dma_start(out=st, in_=src)
            nc.vector.tensor_copy(dst, st)

        # ------------- attention -------------
        with (
            tc.tile_pool(name="stg_ps", bufs=2, space="PSUM") as stgps,
            tc.tile_pool(name="uh_ps", bufs=2, space="PSUM") as uhps,
            tc.tile_pool(name="small_ps", bufs=2, space="PSUM") as smallps,
        ):
            pend = None

            def late_stage(p):
                # et / exp / w-cols / context for batch row b (one row late)
                b, tanhG, ann_halves = p
                denb = smallsb.tile([1, NT], F32, tag="den")
                w_cols = smallsb.tile([128, TS, 16], ANT, tag="wcols")
                for i in range(NT):
                    et_ps = smallps.tile([1, TT], F32, tag="sm")
                    for g in range(M // 2):
                        nc.tensor.matmul(
                            et_ps, lhsT=v_pad[:, 2 * g:2 * g + 2, 0:1],
                            rhs=tanhG[:, 2 * g:2 * g + 2,
                                      TT * i:TT * (i + 1)],
                            start=(g == 0), stop=(g == M // 2 - 1),
                            perf_mode=DR)
                    w_row = smallsb.tile([1, TT], ANT, tag="wrow")
                    nc.scalar.activation(w_row, et_ps, AF.Exp,
                                         scale=1.0 / V_SCALE,
                                         accum_out=denb[:, i:i + 1])
                    wc_ps = smallps.tile([128, NS * 4], ANT, tag="sm")
                    wcw = 4 if ANN_FP8 else 2  # pad cols to 4 bytes
                    for s in range(NS):
                        nc.tensor.transpose(wc_ps[:, wcw * s:wcw * s + 1],
                                            w_row[:, 128 * s:128 * (s + 1)],
                                            ones11_t)
                    nc.vector.tensor_copy(
                        w_cols[:, NS * i:NS * (i + 1), 0],
                        wc_ps.rearrange("p (s w) -> p s w", w=wcw)[:, 0:NS, 0])
                dsum = smallsb.tile([1, 1], F32, tag="dsum")
                nc.vector.reduce_sum(dsum, denb, axis=mybir.AxisListType.X)
                drec = smallsb.tile([1, 1], F32, tag="drec")
                nc.vector.reciprocal(drec, dsum)
                ctx_ps = smallps.tile([1, A], F32, tag="sm")
                if ANN_FP8:
                    for h in range(2):
                        for g in range(HQ // 2):
                            nc.tensor.matmul(
                                ctx_ps,
                                lhsT=w_cols[:, HQ * h + 2 * g:
                                            HQ * h + 2 * g + 2, 0:1],
                                rhs=ann_halves[h][:, 2 * g:2 * g + 2, :],
                                start=(h == 0 and g == 0),
                                stop=(h == 1 and g == HQ // 2 - 1),
                                perf_mode=DR)
                else:
                    for q in range(TS):
                        nc.tensor.matmul(
                            ctx_ps, lhsT=w_cols[:, q, 0:1],
                            rhs=ann_halves[q // HQ][:, q % HQ, :],
                            start=(q == 0), stop=(q == TS - 1))
                ctx_row = smallsb.tile([1, A], F32, tag="ctxrow")
                nc.vector.tensor_scalar_mul(ctx_row, ctx_ps, drec)
                cT_ps = smallps.tile([128, J], F32, tag="sm")
                for j in range(J):
                    nc.tensor.transpose(cT_ps[:, j:j + 1],
                                        ctx_row[:, 128 * j:128 * (j + 1)],
                                        ones11)
                nc.vector.tensor_copy(xT[:, J:2 * J, b], cT_ps)

            for b in range(BS):
                ann_halves = ann_q.pop(0)
                if b + 2 < BS:
                    ann_q.append(issue_ann_dma(b + 2))
                for c in (2 * b, 2 * b + 1):
                    if c < len(w_chunks):
                        issue_w_chunk(c)

                # Per half-row: transpose burst then uh/tanh burst, so plain
                # matmuls pulse on PE at a sub-3.4us cadence (HAM warmth).
                # Transposes are REGULAR bf16 matmuls (out = ann_chunk.T @ I)
                # rather than transpose-mode, which the HAM activity monitor
                # does not count as PE-busy.
                annT = annTpool.tile([128, J, T], FP8)
                tanhG = tanhpool.tile([128, M, T], FP8)
                for h in range(2):
                    for i in range(HQ // NS):
                        for j in range(J):
                            stg = stgps.tile([128, TT], F32, tag="stg")
                            for s in range(NS):
                                nc.tensor.matmul(
                                    stg[:, 128 * s:128 * (s + 1)],
                                    lhsT=ann_halves[h][:, NS * i + s,
                                                       128 * j:128 * (j + 1)],
                                    rhs=ident_t,
                                    start=True, stop=True)
                            nc.vector.tensor_copy(
                                annT[:, j, T // 2 * h + TT * i:
                                     T // 2 * h + TT * (i + 1)], stg)
                    for m in range(M):
                        uh = uhps.tile([128, HB], F32, tag="uh")
                        for c in range(2):
                            for g in range(J // 2):
                                nc.tensor.matmul(
                                    uh[:, TT * c:TT * (c + 1)],
                                    lhsT=ku_sb[:, 2 * g:2 * g + 2,
                                               128 * m:128 * (m + 1)],
                                    rhs=annT[:, 2 * g:2 * g + 2,
                                             HB * h + TT * c:
                                             HB * h + TT * (c + 1)],
                                    start=(g == 0), stop=(g == J // 2 - 1),
                                    perf_mode=DR)
                        nc.scalar.activation(
                            tanhG[:, m, HB * h:HB * (h + 1)], uh, AF.Tanh,
                            bias=bias_att[:, m, b:b + 1],
                            scale=1.0 / KU_SCALE)
                    if h == 0 and pend is not None:
                        late_stage(pend)
                        pend = None

                pend = (b, tanhG, ann_halves)

            late_stage(pend)

        # ------------- LSTM tail -------------
        with (
            tc.tile_pool(name="z_ps", bufs=2, space="PSUM") as zpool,
            tc.tile_pool(name="gates", bufs=1) as gpool,
        ):
            gates = []
            for n in range(4):
                zps = zpool.tile([BS, U], F32)
                for k in range(2 * J):
                    nc.tensor.matmul(zps, lhsT=xT[:, k, :],
                                     rhs=Wt[:, k, U * n:U * (n + 1)],
                                     start=(k == 0), stop=False)
                for k in range(M):
                    nc.tensor.matmul(zps, lhsT=hT[:, k, :],
                                     rhs=Rt[:, k, U * n:U * (n + 1)],
                                     start=False, stop=False)
                nc.tensor.matmul(zps, lhsT=ones1b,
                                 rhs=biasz_row[:, U * n:U * (n + 1)],
                                 start=False, stop=True)
                g = gpool.tile([BS, U], F32, tag=f"gate{n}")
                if n == 2:
                    nc.scalar.activation(g, zps, AF.Tanh)
                else:
                    nc.scalar.activation(g, zps, AF.Relu, bias=half_col,
                                         scale=0.2)
                    nc.vector.tensor_scalar_min(g, g, 1.0)
                gates.append(g)

            gi, gf, gg, go = gates
            c_new = gpool.tile([BS, U], F32, tag="cnew")
            nc.vector.tensor_mul(c_new, gf, c_nat)
            ig = gpool.tile([BS, U], F32, tag="ig")
            nc.vector.tensor_mul(ig, gi, gg)
            nc.vector.tensor_add(c_new, c_new, ig)
            tc_t = gpool.tile([BS, U], F32, tag="tanhc")
            nc.scalar.activation(tc_t, c_new, AF.Tanh)
            h_new = gpool.tile([BS, U], F32, tag="hnew")
            nc.vector.tensor_mul(h_new, go, tc_t)
            nc.sync.dma_start(out=out_d, in_=h_new)


_NC_CACHE = None


def _get_nc():
    global _NC_CACHE
    if _NC_CACHE is None:
        _NC_CACHE = build_bass()
    return _NC_CACHE


def make_in_maps(inputs, h, c, annotations, kernel, recurrent_kernel, bias,
                 kernel_u, kernel_w, kernel_v):
    asc = np.ascontiguousarray
    maps = []
    for core in range(N_CORES):
        sl = slice(core * BS, (core + 1) * BS)
        maps.append({
            "ann": asc(annotations[sl]).astype(np.float32),
            "inputs": asc(inputs[sl]).astype(np.float32),
            "h": asc(h[sl]).astype(np.float32),
            "c": asc(c[sl]).astype(np.float32),
            "kernel": asc(kernel).astype(np.float32),
            "rkernel": asc(recurrent_kernel).astype(np.float32),
            "bias": asc(bias).reshape(1, 6 * U).astype(np.float32),
            "ku": asc(kernel_u).astype(np.float32),
            "kw": asc(kernel_w).astype(np.float32),
            "kv": asc(kernel_v).reshape(1, U).astype(np.float32),
        })
    return maps


def kernel(inputs, h, c, annotations, kernel, recurrent_kernel, bias,
           kernel_u, kernel_w, kernel_v, _trace=False):
    nc = _get_nc()
    in_maps = make_in_maps(inputs, h, c, annotations, kernel,
                           recurrent_kernel, bias, kernel_u, kernel_w,
                           kernel_v)
    res = run_bass_kernel_spmd(nc, in_maps, list(range(N_CORES)),
                               trace=_trace)
    out = np.concatenate([res.results[i]["out"] for i in range(N_CORES)],
                         axis=0)
    if _trace:
        globals()["last_exec_time_ns"] = res.exec_time_ns
        globals()["last_results"] = res
    return out
